# revision 23
# baseline (speedup 1.0000x reference)
"""Fused attention block (QKV proj -> softmax attention -> out proj -> residual+LN)
for B=4, S=2048, D=512, H=8, DH=64 on 8 TRN2 NeuronCores.

Sharding: token-parallel. Core c handles batch b=c//2, query tokens
[(c%2)*1024, (c%2+1)*1024) of that batch. Each core redundantly computes K/V
for its batch's full 2048-token sequence (cheaper than an AllGather), runs
flash-style attention fully on-chip (the 537MB score tensor never touches HBM),
and writes its own disjoint 1024x512 slice of the output. Zero collectives.

Engine budget: the exp stream on the Scalar(ACT) engine is the throughput
floor (128 x ~1.1us). Everything else is arranged to overlap with it:
- Host permutes x so the core's local 1024 query tokens come first in the
  key order (softmax is key-order invariant) -> single xt input.
- QKV bias folds into the PSUM->SBUF copy on the Vector engine
  (tensor_scalar_add with a per-partition scalar for Q/K, tensor_add with a
  host-broadcast row for V).
- Softmax denominators (row 64 of each ctx accumulation) are stashed to SBUF,
  gathered by tiny SBUF->SBUF DMAs into one [16,512] tile, and inverted with
  two exact DVE reciprocals; the reciprocal row is broadcast across 64
  partitions with a K=1 ones matmul into PSUM (no GPSIMD broadcast).
- V projection is interleaved into head 0's attention loop and later K/Q
  projections into subsequent heads so exp starts ~8us in and stays dense.
- xres and the output travel as bf16.
"""

import os
import sys

import numpy as np

for _p in ("/opt/trn_rl_repo",):
    if os.path.isdir(_p) and _p not in sys.path:
        sys.path.insert(0, _p)

import ml_dtypes

import concourse.bacc as bacc
import concourse.bass as bass
import concourse.tile as tile
from concourse import mybir
from concourse.bass_utils import run_bass_kernel_spmd

BF16 = mybir.dt.bfloat16
F32 = mybir.dt.float32
AF = mybir.ActivationFunctionType
ALU = mybir.AluOpType

P = 128        # partitions
D = 512        # hidden dim
DH = 64        # head dim
H = 8          # heads
S = 2048       # tokens per batch element
TQ = 1024      # query tokens per core
B = 4
NCORES = 8
EPS = 1e-5

TRACE = False
LAST_RESULTS = None
_NC_CACHE = None


def _build():
    nc = bacc.Bacc()

    # x[b] permuted so local query tokens are first, then transposed: [D, S]
    xt = nc.declare_dram_parameter("xt", [D, S], BF16, isOutput=False)
    xres = nc.declare_dram_parameter("xres", [TQ, D], BF16, isOutput=False)  # x_local + bo
    wqt = nc.declare_dram_parameter("wqt", [D, D], BF16, isOutput=False)     # Wq.T
    wkt = nc.declare_dram_parameter("wkt", [D, D], BF16, isOutput=False)
    wvt = nc.declare_dram_parameter("wvt", [D, D], BF16, isOutput=False)
    wot = nc.declare_dram_parameter("wot", [D, D], BF16, isOutput=False)
    bqp = nc.declare_dram_parameter("bq", [P, 4], F32, isOutput=False)   # bq.reshape(4,128).T
    bkp = nc.declare_dram_parameter("bk", [P, 4], F32, isOutput=False)
    bvp = nc.declare_dram_parameter("bv", [P, D], BF16, isOutput=False)  # host-broadcast
    outp = nc.declare_dram_parameter("out", [TQ, D], BF16, isOutput=True)

    with tile.TileContext(nc) as tc:
        with (
            tc.tile_pool(name="big", bufs=1) as big,
            tc.tile_pool(name="work", bufs=3) as work,
            tc.tile_pool(name="ps_st", bufs=2, space="PSUM") as ps_st,
            tc.tile_pool(name="ps_ctx", bufs=2, space="PSUM") as ps_ctx,
            tc.tile_pool(name="ps_mm", bufs=2, space="PSUM") as ps_mm,
        ):
            # ---------------- loads (priority order) ----------------
            wk_sb = big.tile([P, 4, D], BF16)
            xt_sb = big.tile([P, 4, S], BF16)
            wq_sb = big.tile([P, 4, D], BF16)
            wv_sb = big.tile([P, 4, D], BF16)
            wo_sb = big.tile([P, 4, D], BF16)
            bq_sb = big.tile([P, 4], F32)
            bk_sb = big.tile([P, 4], F32)
            bv_sb = big.tile([P, D], BF16)
            nc.sync.dma_start(out=bq_sb[:, :], in_=bqp[:, :])
            nc.sync.dma_start(out=bk_sb[:, :], in_=bkp[:, :])
            # weights split by m-column so m=0 (head-pair 0) lands first
            for c in range(4):
                nc.sync.dma_start(out=wk_sb[:, c, 0:P], in_=wkt[c * P:(c + 1) * P, 0:P])
            # xt in [kc, t4] chunks, t4-major so K/Q of m=0 unblock first
            for t4 in range(2):
                for c in range(4):
                    nc.sync.dma_start(
                        out=xt_sb[:, c, t4 * D:(t4 + 1) * D],
                        in_=xt[c * P:(c + 1) * P, t4 * D:(t4 + 1) * D],
                    )
            for c in range(4):
                nc.sync.dma_start(out=wq_sb[:, c, 0:P], in_=wqt[c * P:(c + 1) * P, 0:P])
            nc.sync.dma_start(out=bv_sb[:, :], in_=bvp[:, :])
            for c in range(4):
                nc.sync.dma_start(out=wv_sb[:, c, :], in_=wvt[c * P:(c + 1) * P, :])
            for t4 in range(2, 4):
                for c in range(4):
                    nc.sync.dma_start(
                        out=xt_sb[:, c, t4 * D:(t4 + 1) * D],
                        in_=xt[c * P:(c + 1) * P, t4 * D:(t4 + 1) * D],
                    )
            for c in range(4):
                nc.sync.dma_start(out=wk_sb[:, c, P:4 * P], in_=wkt[c * P:(c + 1) * P, P:4 * P])
                nc.sync.dma_start(out=wq_sb[:, c, P:4 * P], in_=wqt[c * P:(c + 1) * P, P:4 * P])
            for c in range(4):
                nc.sync.dma_start(out=wo_sb[:, c, :], in_=wot[c * P:(c + 1) * P, :])
            xres_sb = big.tile([P, 8, D], BF16)
            for i in range(8):
                nc.sync.dma_start(out=xres_sb[:, i, :], in_=xres[i * P:(i + 1) * P, :])

            ones_sb = big.tile([1, 64], BF16)
            nc.vector.memset(ones_sb[:, :], 1.0)
            eps_sb = big.tile([P, 1], F32)
            nc.vector.memset(eps_sb[:, :], EPS)

            # V augmented with a ones column per head: [tok, (h, 64 dims + 1)]
            vaug = big.tile([P, 16, H * 65], BF16)
            nc.vector.memset(
                vaug[:, :, :].rearrange("p c (h e) -> p c h e", e=65)[:, :, :, 64:65],
                1.0,
            )

            qt_all = big.tile([P, 4, TQ], BF16)   # Q^T  [dq, tq]
            kt_all = big.tile([P, 4, S], BF16)    # K^T  [dk, t]
            qt_dup = big.tile([P, 4, TQ], BF16)   # partition-swapped copy of Q^T
            kt_dup = big.tile([P, 4, S], BF16)    # partition-swapped copy of K^T
            ctxT = big.tile([P, 4, TQ], BF16)     # normalized ctx^T [dv, tq]
            raw_ctx = big.tile([P, 16, 512], BF16)  # unnormalized ctx + den row
            rec_flat = big.tile([1, 16, 512], BF16)  # partition-0 recips for matmul rhs
            GROUPS = [list(range(0, 8)), list(range(8, 14)), list(range(14, 16))]
            den_g = [big.tile([len(g), 512], BF16, name=f"den_g{i}")
                     for i, g in enumerate(GROUPS)]
            y_all = big.tile([P, 8, D], F32)      # proj + residual
            mv_all = big.tile([P, 8, 2], F32)     # (mean, var) per token tile
            rstd_all = big.tile([P, 8], F32)

            # ---------------- projection emitters ----------------
            def k_proj(m, t4):
                ps = ps_mm.tile([P, D], F32, tag="mm", name="ps_k")
                for kc in range(4):
                    nc.tensor.matmul(
                        ps[:, :],
                        lhsT=wk_sb[:, kc, m * P:(m + 1) * P],
                        rhs=xt_sb[:, kc, t4 * D:(t4 + 1) * D],
                        start=(kc == 0),
                        stop=(kc == 3),
                    )
                nc.vector.tensor_scalar_add(
                    kt_all[:, m, t4 * D:(t4 + 1) * D], ps[:, :], bk_sb[:, m:m + 1]
                )
                sl = slice(t4 * D, (t4 + 1) * D)
                nc.sync.dma_start(out=kt_dup[64:128, m, sl], in_=kt_all[0:64, m, sl])
                nc.sync.dma_start(out=kt_dup[0:64, m, sl], in_=kt_all[64:128, m, sl])

            def q_proj(m, t2):
                ps = ps_mm.tile([P, D], F32, tag="mm", name="ps_q")
                for kc in range(4):
                    nc.tensor.matmul(
                        ps[:, :],
                        lhsT=wq_sb[:, kc, m * P:(m + 1) * P],
                        rhs=xt_sb[:, kc, t2 * D:(t2 + 1) * D],
                        start=(kc == 0),
                        stop=(kc == 3),
                    )
                nc.vector.tensor_scalar_add(
                    qt_all[:, m, t2 * D:(t2 + 1) * D], ps[:, :], bq_sb[:, m:m + 1]
                )
                sl = slice(t2 * D, (t2 + 1) * D)
                nc.sync.dma_start(out=qt_dup[64:128, m, sl], in_=qt_all[0:64, m, sl])
                nc.sync.dma_start(out=qt_dup[0:64, m, sl], in_=qt_all[64:128, m, sl])

            def v_proj(t16):
                ps = ps_mm.tile([P, D], F32, tag="mm", name="ps_v")
                for kc in range(4):
                    nc.tensor.matmul(
                        ps[:, :],
                        lhsT=xt_sb[:, kc, t16 * P:(t16 + 1) * P],
                        rhs=wv_sb[:, kc, :],
                        start=(kc == 0),
                        stop=(kc == 3),
                    )
                nc.vector.tensor_add(
                    out=vaug[:, t16, :].rearrange("p (h e) -> p h e", e=65)[:, :, 0:64],
                    in0=ps[:, :].rearrange("p (h e) -> p h e", e=64),
                    in1=bv_sb[:, :].rearrange("p (h e) -> p h e", e=64),
                )

            # K/Q for head-pair 0 up front (first half of keys); the rest of
            # K0 is interleaved into head 0 just-in-time.
            for t4 in range(2):
                k_proj(0, t4)
            for t2 in range(2):
                q_proj(0, t2)

            # per-head interleave schedule: head -> {kc: [units]}
            inter = {h: {} for h in range(H)}
            for kc in range(16):
                inter[0][kc] = [lambda t16=kc: v_proj(t16)]
            inter[0][4].append(lambda: k_proj(0, 2))
            inter[0][8].append(lambda: k_proj(0, 3))
            units1 = [lambda t4=t4: k_proj(1, t4) for t4 in range(4)] + [
                lambda t2=t2: q_proj(1, t2) for t2 in range(2)
            ]
            for i, u in enumerate(units1):
                inter[1].setdefault(i * 2, []).append(u)
            units2 = [lambda t4=t4: k_proj(2, t4) for t4 in range(4)] + [
                lambda t2=t2: q_proj(2, t2) for t2 in range(2)
            ]
            for i, u in enumerate(units2):
                inter[2 + i % 2].setdefault((i // 2) * 5, []).append(u)
            units3 = [lambda t4=t4: k_proj(3, t4) for t4 in range(4)] + [
                lambda t2=t2: q_proj(3, t2) for t2 in range(2)
            ]
            for i, u in enumerate(units3):
                inter[4 + i % 2].setdefault((i // 2) * 5, []).append(u)

            # phase A of the output projection (head-pairs 0,1), spread
            # across head 4's chunk loop to avoid a tensor/DVE burst
            def phase_a(t8):
                ps = ps_mm.tile([P, D], F32, tag="mm", name="ps_oa")
                for c in range(2):
                    nc.tensor.matmul(
                        ps[:, :],
                        lhsT=ctxT[:, c, t8 * P:(t8 + 1) * P],
                        rhs=wo_sb[:, c, :],
                        start=(c == 0),
                        stop=(c == 1),
                    )
                nc.vector.tensor_add(
                    out=y_all[:, t8, :], in0=ps[:, :], in1=xres_sb[:, t8, :]
                )
            for t8 in range(8):
                inter[4].setdefault(t8 * 2 + 1, []).append(
                    lambda t8=t8: phase_a(t8))

            # normalize a group of stashed (h, qt2) slots: one reciprocal for
            # the group, then per slot a K=1 ones-matmul broadcast + multiply
            def normalize_group(g):
                n = len(GROUPS[g])
                den_f = work.tile([8, 512], F32, tag="denf")
                nc.vector.tensor_copy(den_f[0:n, :], den_g[g][:, :])
                rec_f = work.tile([8, 512], F32, tag="recf")
                nc.vector.reciprocal(rec_f[0:n, :], den_f[0:n, :])
                rec_c = work.tile([8, 512], BF16, tag="recc")
                nc.vector.tensor_copy(rec_c[0:n, :], rec_f[0:n, :])
                for j, s in enumerate(GROUPS[g]):
                    nc.sync.dma_start(
                        out=rec_flat[0:1, s, :], in_=rec_c[j:j + 1, :]
                    )
                for s in GROUPS[g]:
                    h, qt2 = s // 2, s % 2
                    po = (h % 2) * 64
                    chn = h // 2
                    rb = ps_mm.tile([P, 512], F32, tag="mm", name="ps_rb")
                    nc.tensor.matmul(
                        rb[0:64, :],
                        lhsT=ones_sb[0:1, :],
                        rhs=rec_flat[0:1, s, :],
                        start=True,
                        stop=True,
                    )
                    nc.vector.tensor_mul(
                        out=ctxT[po:po + 64, chn, qt2 * 512:(qt2 + 1) * 512],
                        in0=raw_ctx[0:64, s, :],
                        in1=rb[0:64, :],
                    )

            # ---------------- attention ----------------
            for h in range(H):
                po = (h % 2) * 64
                chn = h // 2
                cx0 = ps_ctx.tile([65, 512], F32, tag="cx")
                cx1 = ps_ctx.tile([65, 512], F32, tag="cx")
                for kc in range(16):
                    for u in inter[h].get(kc, []):
                        u()
                    st = ps_st.tile([P, TQ], F32, tag="st")
                    dpo = 64 - po
                    nc.tensor.matmul(
                        st[:, 0:512],
                        lhsT=kt_all[po:po + 64, chn, kc * P:(kc + 1) * P],
                        rhs=qt_all[po:po + 64, chn, 0:512],
                        start=True,
                        stop=True,
                    )
                    if h == 0:
                        # ramp: avoid the dup-copy dependency so the exp
                        # stream starts as soon as K0/Q0 land
                        nc.tensor.matmul(
                            st[:, 512:1024],
                            lhsT=kt_all[po:po + 64, chn, kc * P:(kc + 1) * P],
                            rhs=qt_all[po:po + 64, chn, 512:1024],
                            start=True,
                            stop=True,
                        )
                    else:
                        nc.tensor.matmul(
                            st[:, 512:1024],
                            lhsT=kt_dup[dpo:dpo + 64, chn, kc * P:(kc + 1) * P],
                            rhs=qt_dup[dpo:dpo + 64, chn, 512:1024],
                            start=True,
                            stop=True,
                        )
                    pr = work.tile([P, TQ], BF16, tag="probs")
                    nc.scalar.activation(
                        out=pr[:, :], in_=st[:, :], func=AF.Exp, scale=0.125
                    )
                    vh = vaug[:, kc, h * 65:(h + 1) * 65]
                    nc.tensor.matmul(
                        cx0[:, :], lhsT=vh, rhs=pr[:, 0:512],
                        start=(kc == 0), stop=(kc == 15),
                    )
                    nc.tensor.matmul(
                        cx1[:, :], lhsT=vh, rhs=pr[:, 512:1024],
                        start=(kc == 0), stop=(kc == 15),
                    )
                # stash unnormalized ctx (+ denominator row 64) and gather dens
                for qt2, cx in ((0, cx0), (1, cx1)):
                    s = 2 * h + qt2
                    nc.vector.tensor_copy(raw_ctx[0:65, s, :], cx[0:65, :])
                    gi = 0 if s < 8 else (1 if s < 14 else 2)
                    base = GROUPS[gi][0]
                    nc.sync.dma_start(
                        out=den_g[gi][s - base:s - base + 1, :],
                        in_=raw_ctx[64:65, s, :],
                    )
                if h == 3:
                    normalize_group(0)
                if h == 6:
                    normalize_group(1)
            normalize_group(2)

            # ------- out proj phase B + per-tile LN (gamma/beta on host)
            for t8 in range(8):
                ps = ps_mm.tile([P, D], F32, tag="mm", name="ps_o")
                for c in range(2, 4):
                    nc.tensor.matmul(
                        ps[:, :],
                        lhsT=ctxT[:, c, t8 * P:(t8 + 1) * P],
                        rhs=wo_sb[:, c, :],
                        start=(c == 2),
                        stop=(c == 3),
                    )
                nc.vector.tensor_add(
                    out=y_all[:, t8, :], in0=ps[:, :], in1=y_all[:, t8, :]
                )
                stt = work.tile([P, 6], F32, tag="bnst")
                nc.vector.bn_stats(out=stt[:, :], in_=y_all[:, t8, :])
                nc.vector.bn_aggr(out=mv_all[:, t8, :], in_=stt[:, :])
                std1 = work.tile([P, 1], F32, tag="std1")
                nc.scalar.activation(
                    out=std1[:, :],
                    in_=mv_all[:, t8, 1:2],
                    func=AF.Sqrt,
                    bias=eps_sb[:, :],
                    scale=1.0,
                )
                nc.vector.reciprocal(rstd_all[:, t8:t8 + 1], std1[:, :])
                fin = work.tile([P, D], BF16, tag="lnfin")
                nc.vector.tensor_scalar(
                    out=fin[:, :],
                    in0=y_all[:, t8, :],
                    scalar1=mv_all[:, t8, 0:1],
                    scalar2=rstd_all[:, t8:t8 + 1],
                    op0=ALU.subtract,
                    op1=ALU.mult,
                )
                nc.sync.dma_start(out=outp[t8 * P:(t8 + 1) * P, :], in_=fin[:, :])

    nc.compile()
    return nc


def _get_nc():
    global _NC_CACHE
    if _NC_CACHE is None:
        _NC_CACHE = _build()
    return _NC_CACHE


def kernel(x, Wq, bq, Wk, bk, Wv, bv, Wo, bo, gamma, beta):
    global LAST_RESULTS
    bf = ml_dtypes.bfloat16
    x = np.asarray(x, np.float32)
    bo = np.asarray(bo, np.float32)
    wqt_n = np.ascontiguousarray(np.asarray(Wq, np.float32).T).astype(bf)
    wkt_n = np.ascontiguousarray(np.asarray(Wk, np.float32).T).astype(bf)
    wvt_n = np.ascontiguousarray(np.asarray(Wv, np.float32).T).astype(bf)
    wot_n = np.ascontiguousarray(np.asarray(Wo, np.float32).T).astype(bf)
    bq_n = np.ascontiguousarray(np.asarray(bq, np.float32).reshape(4, P).T)
    bk_n = np.ascontiguousarray(np.asarray(bk, np.float32).reshape(4, P).T)
    bv_n = np.ascontiguousarray(
        np.broadcast_to(np.asarray(bv, np.float32)[None, :], (P, D))).astype(bf)

    in_maps = []
    for c in range(NCORES):
        b = c // 2
        par = c % 2
        xb = x[b]                               # [S, D]
        xloc = xb[par * TQ:(par + 1) * TQ]      # [TQ, D]
        xoth = xb[(1 - par) * TQ:(2 - par) * TQ]
        xperm = np.concatenate([xloc, xoth], axis=0)   # local queries first
        in_maps.append({
            "xt": np.ascontiguousarray(xperm.T).astype(bf),
            "xres": np.ascontiguousarray(xloc + bo[None, :]).astype(bf),
            "wqt": wqt_n, "wkt": wkt_n, "wvt": wvt_n, "wot": wot_n,
            "bq": bq_n, "bk": bk_n, "bv": bv_n,
        })

    nc = _get_nc()
    res = run_bass_kernel_spmd(nc, in_maps, core_ids=list(range(NCORES)), trace=TRACE)
    LAST_RESULTS = res

    outf = np.empty((B, S, D), np.float32)
    for c in range(NCORES):
        b = c // 2
        par = c % 2
        outf[b, par * TQ:(par + 1) * TQ, :] = np.asarray(
            res.results[c]["out"], dtype=np.float32)
    gm = np.asarray(gamma, np.float32)[None, None, :]
    bt = np.asarray(beta, np.float32)[None, None, :]
    return outf * gm + bt


# revision 24
# speedup vs baseline: 1.1981x; 1.1981x over previous
"""Fused attention block (QKV proj -> softmax attention -> out proj -> residual+LN)
for B=4, S=2048, D=512, H=8, DH=64 on 8 TRN2 NeuronCores.

Sharding: token-parallel. Core c handles batch b=c//2, query tokens
[(c%2)*1024, (c%2+1)*1024) of that batch. Each core redundantly computes K/V
for its batch's full 2048-token sequence (cheaper than an AllGather), runs
flash-style attention fully on-chip (the 537MB score tensor never touches HBM),
and writes its own disjoint 1024x512 slice of the output. Zero collectives.

Engine budget: the exp stream on the Scalar(ACT) engine is the throughput
floor (128 x ~1.1us). Everything else is arranged to overlap with it:
- Host permutes x so the core's local 1024 query tokens come first in the
  key order (softmax is key-order invariant) -> single xt input.
- QKV bias folds into the PSUM->SBUF copy on the Vector engine
  (tensor_scalar_add with a per-partition scalar for Q/K, tensor_add with a
  host-broadcast row for V).
- Softmax denominators (row 64 of each ctx accumulation) are stashed to SBUF,
  gathered by tiny SBUF->SBUF DMAs into one [16,512] tile, and inverted with
  two exact DVE reciprocals; the reciprocal row is broadcast across 64
  partitions with a K=1 ones matmul into PSUM (no GPSIMD broadcast).
- V projection is interleaved into head 0's attention loop and later K/Q
  projections into subsequent heads so exp starts ~8us in and stays dense.
- xres and the output travel as bf16.
"""

import os
import sys

import numpy as np

for _p in ("/opt/trn_rl_repo",):
    if os.path.isdir(_p) and _p not in sys.path:
        sys.path.insert(0, _p)

import ml_dtypes

import concourse.bacc as bacc
import concourse.bass as bass
import concourse.tile as tile
from concourse import mybir
from concourse.bass_utils import run_bass_kernel_spmd

BF16 = mybir.dt.bfloat16
F32 = mybir.dt.float32
AF = mybir.ActivationFunctionType
ALU = mybir.AluOpType

P = 128        # partitions
D = 512        # hidden dim
DH = 64        # head dim
H = 8          # heads
S = 2048       # tokens per batch element
TQ = 1024      # query tokens per core
B = 4
NCORES = 8
EPS = 1e-5

TRACE = False
LAST_RESULTS = None
_NC_CACHE = None


def _build():
    nc = bacc.Bacc()

    # x[b] permuted so local query tokens are first, then transposed: [D, S]
    xt = nc.declare_dram_parameter("xt", [D, S], BF16, isOutput=False)
    xres = nc.declare_dram_parameter("xres", [TQ, D], BF16, isOutput=False)  # x_local + bo
    wqt = nc.declare_dram_parameter("wqt", [D, D], BF16, isOutput=False)     # Wq.T
    wkt = nc.declare_dram_parameter("wkt", [D, D], BF16, isOutput=False)
    wvt = nc.declare_dram_parameter("wvt", [D, D], BF16, isOutput=False)
    wot = nc.declare_dram_parameter("wot", [D, D], BF16, isOutput=False)
    bqp = nc.declare_dram_parameter("bq", [P, 4], F32, isOutput=False)   # bq.reshape(4,128).T
    bkp = nc.declare_dram_parameter("bk", [P, 4], F32, isOutput=False)
    bvp = nc.declare_dram_parameter("bv", [P, D], BF16, isOutput=False)  # host-broadcast
    outp = nc.declare_dram_parameter("out", [TQ, D], BF16, isOutput=True)

    with tile.TileContext(nc) as tc:
        with (
            tc.tile_pool(name="big", bufs=1) as big,
            tc.tile_pool(name="work", bufs=3) as work,
            tc.tile_pool(name="ps_st", bufs=2, space="PSUM") as ps_st,
            tc.tile_pool(name="ps_ctx", bufs=2, space="PSUM") as ps_ctx,
            tc.tile_pool(name="ps_mm", bufs=2, space="PSUM") as ps_mm,
        ):
            # ---------------- loads (priority order) ----------------
            wk_sb = big.tile([P, 4, D], BF16)
            xt_sb = big.tile([P, 4, S], BF16)
            wq_sb = big.tile([P, 4, D], BF16)
            wv_sb = big.tile([P, 4, D], BF16)
            wo_sb = big.tile([P, 4, D], BF16)
            bq_sb = big.tile([P, 4], F32)
            bk_sb = big.tile([P, 4], F32)
            bv_sb = big.tile([P, D], BF16)
            nc.sync.dma_start(out=bq_sb[:, :], in_=bqp[:, :])
            nc.sync.dma_start(out=bk_sb[:, :], in_=bkp[:, :])
            # weights split by m-column so m=0 (head-pair 0) lands first
            for c in range(4):
                nc.sync.dma_start(out=wk_sb[:, c, 0:P], in_=wkt[c * P:(c + 1) * P, 0:P])
            # xt in [kc, t4] chunks, t4-major so K/Q of m=0 unblock first
            for t4 in range(2):
                for c in range(4):
                    nc.sync.dma_start(
                        out=xt_sb[:, c, t4 * D:(t4 + 1) * D],
                        in_=xt[c * P:(c + 1) * P, t4 * D:(t4 + 1) * D],
                    )
            for c in range(4):
                nc.sync.dma_start(out=wq_sb[:, c, 0:P], in_=wqt[c * P:(c + 1) * P, 0:P])
            nc.sync.dma_start(out=bv_sb[:, :], in_=bvp[:, :])
            for c in range(4):
                nc.sync.dma_start(out=wv_sb[:, c, :], in_=wvt[c * P:(c + 1) * P, :])
            for t4 in range(2, 4):
                for c in range(4):
                    nc.sync.dma_start(
                        out=xt_sb[:, c, t4 * D:(t4 + 1) * D],
                        in_=xt[c * P:(c + 1) * P, t4 * D:(t4 + 1) * D],
                    )
            for c in range(4):
                nc.sync.dma_start(out=wk_sb[:, c, P:4 * P], in_=wkt[c * P:(c + 1) * P, P:4 * P])
                nc.sync.dma_start(out=wq_sb[:, c, P:4 * P], in_=wqt[c * P:(c + 1) * P, P:4 * P])
            for c in range(4):
                nc.sync.dma_start(out=wo_sb[:, c, :], in_=wot[c * P:(c + 1) * P, :])
            xres_sb = big.tile([P, 8, D], BF16)
            for i in range(8):
                nc.sync.dma_start(out=xres_sb[:, i, :], in_=xres[i * P:(i + 1) * P, :])

            ones_sb = big.tile([1, 64], BF16)
            nc.vector.memset(ones_sb[:, :], 1.0)
            eps_sb = big.tile([P, 1], F32)
            nc.vector.memset(eps_sb[:, :], EPS)

            # V augmented with a ones column per head: [tok, (h, 64 dims + 1)]
            vaug = big.tile([P, 16, H * 65], BF16)
            nc.vector.memset(
                vaug[:, :, :].rearrange("p c (h e) -> p c h e", e=65)[:, :, :, 64:65],
                1.0,
            )

            qt_all = big.tile([P, 4, TQ], BF16)   # Q^T  [dq, tq]
            kt_all = big.tile([P, 4, S], BF16)    # K^T  [dk, t]
            qt_dup = big.tile([P, 4, TQ], BF16)   # partition-swapped copy of Q^T
            kt_dup = big.tile([P, 4, S], BF16)    # partition-swapped copy of K^T
            ctxT = big.tile([P, 4, TQ], BF16)     # normalized ctx^T [dv, tq]
            raw_ctx = big.tile([P, 16, 512], BF16)  # unnormalized ctx + den row
            rec_flat = big.tile([1, 16, 512], BF16)  # partition-0 recips for matmul rhs
            GROUPS = [list(range(0, 8)), list(range(8, 14)), list(range(14, 16))]
            den_g = [big.tile([len(g), 512], BF16, name=f"den_g{i}")
                     for i, g in enumerate(GROUPS)]
            y_all = big.tile([P, 8, D], F32)      # proj + residual
            mv_all = big.tile([P, 8, 2], F32)     # (mean, var) per token tile
            rstd_all = big.tile([P, 8], F32)

            # ---------------- projection emitters ----------------
            def k_proj(m, t4):
                ps = ps_mm.tile([P, D], F32, tag="mm", name="ps_k")
                for kc in range(4):
                    nc.tensor.matmul(
                        ps[:, :],
                        lhsT=wk_sb[:, kc, m * P:(m + 1) * P],
                        rhs=xt_sb[:, kc, t4 * D:(t4 + 1) * D],
                        start=(kc == 0),
                        stop=(kc == 3),
                    )
                nc.vector.tensor_scalar_add(
                    kt_all[:, m, t4 * D:(t4 + 1) * D], ps[:, :], bk_sb[:, m:m + 1]
                )
                sl = slice(t4 * D, (t4 + 1) * D)
                nc.sync.dma_start(out=kt_dup[64:128, m, sl], in_=kt_all[0:64, m, sl])
                nc.sync.dma_start(out=kt_dup[0:64, m, sl], in_=kt_all[64:128, m, sl])

            def q_proj(m, t2):
                ps = ps_mm.tile([P, D], F32, tag="mm", name="ps_q")
                for kc in range(4):
                    nc.tensor.matmul(
                        ps[:, :],
                        lhsT=wq_sb[:, kc, m * P:(m + 1) * P],
                        rhs=xt_sb[:, kc, t2 * D:(t2 + 1) * D],
                        start=(kc == 0),
                        stop=(kc == 3),
                    )
                nc.vector.tensor_scalar_add(
                    qt_all[:, m, t2 * D:(t2 + 1) * D], ps[:, :], bq_sb[:, m:m + 1]
                )
                sl = slice(t2 * D, (t2 + 1) * D)
                nc.sync.dma_start(out=qt_dup[64:128, m, sl], in_=qt_all[0:64, m, sl])
                nc.sync.dma_start(out=qt_dup[0:64, m, sl], in_=qt_all[64:128, m, sl])

            def v_proj(t16):
                ps = ps_mm.tile([P, D], F32, tag="mm", name="ps_v")
                for kc in range(4):
                    nc.tensor.matmul(
                        ps[:, :],
                        lhsT=xt_sb[:, kc, t16 * P:(t16 + 1) * P],
                        rhs=wv_sb[:, kc, :],
                        start=(kc == 0),
                        stop=(kc == 3),
                    )
                nc.vector.tensor_add(
                    out=vaug[:, t16, :].rearrange("p (h e) -> p h e", e=65)[:, :, 0:64],
                    in0=ps[:, :].rearrange("p (h e) -> p h e", e=64),
                    in1=bv_sb[:, :].rearrange("p (h e) -> p h e", e=64),
                )

            # K/Q for head-pair 0 up front (first half of keys); the rest of
            # K0 is interleaved into head 0 just-in-time.
            for t4 in range(2):
                k_proj(0, t4)
            for t2 in range(2):
                q_proj(0, t2)

            # per-head interleave schedule: head -> {kc: [units]}
            inter = {h: {} for h in range(H)}
            for kc in range(16):
                inter[0][kc] = [lambda t16=kc: v_proj(t16)]
            inter[0][4].append(lambda: k_proj(0, 2))
            inter[0][8].append(lambda: k_proj(0, 3))
            units1 = [lambda t4=t4: k_proj(1, t4) for t4 in range(4)] + [
                lambda t2=t2: q_proj(1, t2) for t2 in range(2)
            ]
            for i, u in enumerate(units1):
                inter[1].setdefault(i * 2, []).append(u)
            units2 = [lambda t4=t4: k_proj(2, t4) for t4 in range(4)] + [
                lambda t2=t2: q_proj(2, t2) for t2 in range(2)
            ]
            for i, u in enumerate(units2):
                inter[2 + i % 2].setdefault((i // 2) * 5, []).append(u)
            units3 = [lambda t4=t4: k_proj(3, t4) for t4 in range(4)] + [
                lambda t2=t2: q_proj(3, t2) for t2 in range(2)
            ]
            for i, u in enumerate(units3):
                inter[4 + i % 2].setdefault((i // 2) * 5, []).append(u)

            # normalize a group of stashed (h, qt2) slots: one reciprocal for
            # the group, then per slot a K=1 ones-matmul broadcast + multiply
            def normalize_group(g):
                n = len(GROUPS[g])
                den_f = work.tile([8, 512], F32, tag="denf")
                nc.vector.tensor_copy(den_f[0:n, :], den_g[g][:, :])
                rec_f = work.tile([8, 512], F32, tag="recf")
                nc.vector.reciprocal(rec_f[0:n, :], den_f[0:n, :])
                rec_c = work.tile([8, 512], BF16, tag="recc")
                nc.vector.tensor_copy(rec_c[0:n, :], rec_f[0:n, :])
                for j, s in enumerate(GROUPS[g]):
                    nc.sync.dma_start(
                        out=rec_flat[0:1, s, :], in_=rec_c[j:j + 1, :]
                    )
                for s in GROUPS[g]:
                    h, qt2 = s // 2, s % 2
                    po = (h % 2) * 64
                    chn = h // 2
                    rb = ps_mm.tile([P, 512], F32, tag="mm", name="ps_rb")
                    nc.tensor.matmul(
                        rb[0:64, :],
                        lhsT=ones_sb[0:1, :],
                        rhs=rec_flat[0:1, s, :],
                        start=True,
                        stop=True,
                    )
                    nc.vector.tensor_mul(
                        out=ctxT[po:po + 64, chn, qt2 * 512:(qt2 + 1) * 512],
                        in0=raw_ctx[0:64, s, :],
                        in1=rb[0:64, :],
                    )

            # ---------------- attention ----------------
            for h in range(H):
                po = (h % 2) * 64
                chn = h // 2
                cx0 = ps_ctx.tile([65, 512], F32, tag="cx")
                cx1 = ps_ctx.tile([65, 512], F32, tag="cx")
                for kc in range(16):
                    for u in inter[h].get(kc, []):
                        u()
                    st = ps_st.tile([P, TQ], F32, tag="st")
                    dpo = 64 - po
                    nc.tensor.matmul(
                        st[:, 0:512],
                        lhsT=kt_all[po:po + 64, chn, kc * P:(kc + 1) * P],
                        rhs=qt_all[po:po + 64, chn, 0:512],
                        start=True,
                        stop=True,
                    )
                    if h == 0 and kc < 4:
                        # ramp: avoid the dup-copy dependency so the exp
                        # stream starts as soon as K0/Q0 land
                        nc.tensor.matmul(
                            st[:, 512:1024],
                            lhsT=kt_all[po:po + 64, chn, kc * P:(kc + 1) * P],
                            rhs=qt_all[po:po + 64, chn, 512:1024],
                            start=True,
                            stop=True,
                        )
                    else:
                        nc.tensor.matmul(
                            st[:, 512:1024],
                            lhsT=kt_dup[dpo:dpo + 64, chn, kc * P:(kc + 1) * P],
                            rhs=qt_dup[dpo:dpo + 64, chn, 512:1024],
                            start=True,
                            stop=True,
                        )
                    pr = work.tile([P, TQ], BF16, tag="probs")
                    nc.scalar.activation(
                        out=pr[:, :], in_=st[:, :], func=AF.Exp, scale=0.125
                    )
                    vh = vaug[:, kc, h * 65:(h + 1) * 65]
                    nc.tensor.matmul(
                        cx0[:, :], lhsT=vh, rhs=pr[:, 0:512],
                        start=(kc == 0), stop=(kc == 15),
                    )
                    nc.tensor.matmul(
                        cx1[:, :], lhsT=vh, rhs=pr[:, 512:1024],
                        start=(kc == 0), stop=(kc == 15),
                    )
                # stash unnormalized ctx (+ denominator row 64) and gather dens
                for qt2, cx in ((0, cx0), (1, cx1)):
                    s = 2 * h + qt2
                    nc.vector.tensor_copy(raw_ctx[0:65, s, :], cx[0:65, :])
                    gi = 0 if s < 8 else (1 if s < 14 else 2)
                    base = GROUPS[gi][0]
                    nc.sync.dma_start(
                        out=den_g[gi][s - base:s - base + 1, :],
                        in_=raw_ctx[64:65, s, :],
                    )
                if h == 3:
                    normalize_group(0)
                if h == 6:
                    normalize_group(1)
                if h == 3:
                    # phase A of the output projection: head-pairs 0,1 are
                    # normalized; accumulate their contribution into y_all
                    for t8 in range(8):
                        ps = ps_mm.tile([P, D], F32, tag="mm", name="ps_oa")
                        for c in range(2):
                            nc.tensor.matmul(
                                ps[:, :],
                                lhsT=ctxT[:, c, t8 * P:(t8 + 1) * P],
                                rhs=wo_sb[:, c, :],
                                start=(c == 0),
                                stop=(c == 1),
                            )
                        nc.vector.tensor_add(
                            out=y_all[:, t8, :], in0=ps[:, :],
                            in1=xres_sb[:, t8, :]
                        )
            normalize_group(2)

            # ------- out proj phase B + per-tile LN (gamma/beta on host)
            for t8 in range(8):
                ps = ps_mm.tile([P, D], F32, tag="mm", name="ps_o")
                for c in range(2, 4):
                    nc.tensor.matmul(
                        ps[:, :],
                        lhsT=ctxT[:, c, t8 * P:(t8 + 1) * P],
                        rhs=wo_sb[:, c, :],
                        start=(c == 2),
                        stop=(c == 3),
                    )
                nc.vector.tensor_add(
                    out=y_all[:, t8, :], in0=ps[:, :], in1=y_all[:, t8, :]
                )
                stt = work.tile([P, 6], F32, tag="bnst")
                nc.vector.bn_stats(out=stt[:, :], in_=y_all[:, t8, :])
                nc.vector.bn_aggr(out=mv_all[:, t8, :], in_=stt[:, :])
                std1 = work.tile([P, 1], F32, tag="std1")
                nc.scalar.activation(
                    out=std1[:, :],
                    in_=mv_all[:, t8, 1:2],
                    func=AF.Sqrt,
                    bias=eps_sb[:, :],
                    scale=1.0,
                )
                nc.vector.reciprocal(rstd_all[:, t8:t8 + 1], std1[:, :])
                fin = work.tile([P, D], BF16, tag="lnfin")
                nc.vector.tensor_scalar(
                    out=fin[:, :],
                    in0=y_all[:, t8, :],
                    scalar1=mv_all[:, t8, 0:1],
                    scalar2=rstd_all[:, t8:t8 + 1],
                    op0=ALU.subtract,
                    op1=ALU.mult,
                )
                nc.sync.dma_start(out=outp[t8 * P:(t8 + 1) * P, :], in_=fin[:, :])

    nc.compile()
    return nc


def _get_nc():
    global _NC_CACHE
    if _NC_CACHE is None:
        _NC_CACHE = _build()
    return _NC_CACHE


def kernel(x, Wq, bq, Wk, bk, Wv, bv, Wo, bo, gamma, beta):
    global LAST_RESULTS
    bf = ml_dtypes.bfloat16
    x = np.asarray(x, np.float32)
    bo = np.asarray(bo, np.float32)
    wqt_n = np.ascontiguousarray(np.asarray(Wq, np.float32).T).astype(bf)
    wkt_n = np.ascontiguousarray(np.asarray(Wk, np.float32).T).astype(bf)
    wvt_n = np.ascontiguousarray(np.asarray(Wv, np.float32).T).astype(bf)
    wot_n = np.ascontiguousarray(np.asarray(Wo, np.float32).T).astype(bf)
    bq_n = np.ascontiguousarray(np.asarray(bq, np.float32).reshape(4, P).T)
    bk_n = np.ascontiguousarray(np.asarray(bk, np.float32).reshape(4, P).T)
    bv_n = np.ascontiguousarray(
        np.broadcast_to(np.asarray(bv, np.float32)[None, :], (P, D))).astype(bf)

    in_maps = []
    for c in range(NCORES):
        b = c // 2
        par = c % 2
        xb = x[b]                               # [S, D]
        xloc = xb[par * TQ:(par + 1) * TQ]      # [TQ, D]
        xoth = xb[(1 - par) * TQ:(2 - par) * TQ]
        xperm = np.concatenate([xloc, xoth], axis=0)   # local queries first
        in_maps.append({
            "xt": np.ascontiguousarray(xperm.T).astype(bf),
            "xres": np.ascontiguousarray(xloc + bo[None, :]).astype(bf),
            "wqt": wqt_n, "wkt": wkt_n, "wvt": wvt_n, "wot": wot_n,
            "bq": bq_n, "bk": bk_n, "bv": bv_n,
        })

    nc = _get_nc()
    res = run_bass_kernel_spmd(nc, in_maps, core_ids=list(range(NCORES)), trace=TRACE)
    LAST_RESULTS = res

    outf = np.empty((B, S, D), np.float32)
    for c in range(NCORES):
        b = c // 2
        par = c % 2
        outf[b, par * TQ:(par + 1) * TQ, :] = np.asarray(
            res.results[c]["out"], dtype=np.float32)
    gm = np.asarray(gamma, np.float32)[None, None, :]
    bt = np.asarray(beta, np.float32)[None, None, :]
    return outf * gm + bt
